# revision 1
# baseline (speedup 1.0000x reference)
"""Trainium2 Bass kernel for nn_LossRecovery (spatial+temporal channel attention).

Sharding: 16 (b,l) slices over 8 cores, 2 slices/core. Redesigned pipeline:
  - x stays resident in SBUF in ONE layout (c-major); 8MB HBM loads per slice
    eliminated (xt_swap via strided access patterns, x_nat via on-chip
    transpose of the final x2).
  - The temporal-k H/W swap is algebraic: sum_s q2[c,s] k2[d,s] is invariant
    to the s-enumeration as long as both sides use the same order, so the
    k2 conv reads natural xt through a (w,h)-strided AP while q2 stays
    natural.
  - The temporal-value scramble keeps the host-side weight-column permutation
    (dperm) of the previous design, with an l-contiguous v2p row layout.
  - PSUM->SBUF copies (the real elementwise cost) are spread across DVE,
    Act (per-partition-bias fused) and GpSimd; v2p/attn2T are bf16 to halve
    SBUF residency (the temporal path is precision-insensitive).
  - All matmuls run float32r (1 cycle/row at free>=256) via bitcast views.
"""
import numpy as np

import concourse.bass as bass
import concourse.bacc as bacc
import concourse.mybir as mybir
import concourse.tile as tile
from concourse.bass_utils import run_bass_kernel_spmd
from concourse.masks import make_identity

B, L, H, W = 2, 8, 64, 64
C, HW = 256, 4096
FP = mybir.dt.float32
FR = mybir.dt.float32r
BF = mybir.dt.bfloat16
NS512 = HW // 512  # 8

_CACHE = {}

ts = bass.ts
ds = bass.ds
AX = None  # set after mybir import in build


def build_program():
    nc = bacc.Bacc("TRN2", target_bir_lowering=False, debug=False, num_devices=8)
    Act = mybir.ActivationFunctionType
    Alu = mybir.AluOpType

    xt_all = nc.dram_tensor("xt_all", [L, C, HW], FP, kind="ExternalInput")
    wqk_d = nc.dram_tensor("wqk", [C, 512], FP, kind="ExternalInput")
    wv_d = nc.dram_tensor("wv", [C, C], FP, kind="ExternalInput")
    wq2_d = nc.dram_tensor("wq2", [C, C], FP, kind="ExternalInput")
    wk2p_d = nc.dram_tensor("wk2p", [C, C], FP, kind="ExternalInput")
    wv2_d = nc.dram_tensor("wv2", [C, 64], FP, kind="ExternalInput")
    qkb_d = nc.dram_tensor("qk_bias", [128, 512], FP, kind="ExternalInput")
    vb_d = nc.dram_tensor("v_bias", [C, 1], FP, kind="ExternalInput")
    q2b_d = nc.dram_tensor("q2_bias", [128, C], FP, kind="ExternalInput")
    k2b_d = nc.dram_tensor("k2_bias", [128, C], FP, kind="ExternalInput")
    v2b_d = nc.dram_tensor("v2_bias", [64, 1], FP, kind="ExternalInput")
    gam_d = nc.dram_tensor("gammas", [128, 2], FP, kind="ExternalInput")
    ones_d = nc.dram_tensor("ones_row", [1, 128], FP, kind="ExternalInput")
    out_d = nc.dram_tensor("out", [2, C, HW], BF, kind="ExternalOutput")

    with tile.TileContext(nc) as tc:
        with (
            tc.tile_pool(name="const", bufs=1) as cpool,
            tc.tile_pool(name="xres", bufs=1) as xres,
            tc.tile_pool(name="xstream", bufs=3) as xsp,
            tc.tile_pool(name="v2pool", bufs=1) as v2pl,
            tc.tile_pool(name="x1pool", bufs=1) as x1pl,
            tc.tile_pool(name="qkp", bufs=3) as qkp,
            tc.tile_pool(name="vtp", bufs=2) as vtp,
            tc.tile_pool(name="attp", bufs=1) as attp,
            tc.tile_pool(name="x2p", bufs=4) as x2p,
            tc.tile_pool(name="sm", bufs=2) as sm,
            tc.tile_pool(name="ps512", bufs=4, space="PSUM") as ps512,
            tc.tile_pool(name="ps256", bufs=2, space="PSUM") as ps256,
            tc.tile_pool(name="psS", bufs=2, space="PSUM") as psS,
        ):
            # ---- weights / constants (plain fp32 tiles; bitcast FR at use) ----
            wqk = cpool.tile([128, 2, 512], FR, tag="wqk")
            qkb = cpool.tile([128, 512], FP, tag="qkb")
            for cc in range(2):
                nc.sync.dma_start(wqk[:, cc, :], wqk_d[ds(128 * cc, 128), :].bitcast(FR))

            xt = [xres.tile([128, 2, HW], FR, tag=f"xt{j}", name=f"xt{j}")
                  for j in range(2)]
            v2p = [v2pl.tile([128, 2, HW], BF, tag=f"v2p{j}", name=f"v2p{j}")
                   for j in range(2)]
            x1t = x1pl.tile([128, 2, HW], FR, tag="x1t")

            nc.sync.dma_start(
                xt[0][:, :, ds(0, 512)],
                xt_all[0, :, ds(0, 512)].rearrange("(cc p) n -> p cc n", p=128)
                .bitcast(FR))
            nc.sync.dma_start(qkb[:], qkb_d[:])
            for et in range(1, 8):
                nc.sync.dma_start(
                    xt[0][:, :, ds(512 * et, 512)],
                    xt_all[0, :, ds(512 * et, 512)]
                    .rearrange("(cc p) n -> p cc n", p=128).bitcast(FR))
            for qt in range(4):
                nc.sync.dma_start(
                    xt[1][:, :, ds(1024 * qt, 1024)],
                    xt_all[1, :, ds(1024 * qt, 1024)]
                    .rearrange("(cc p) n -> p cc n", p=128).bitcast(FR))

            wv = cpool.tile([128, 2, C], FR, tag="wv")
            nc.sync.dma_start(wv[:], wv_d[:].rearrange("(cc p) n -> p cc n", p=128).bitcast(FR))
            vb = cpool.tile([128, 2, 1], FP, tag="vb")
            nc.sync.dma_start(vb[:], vb_d[:].rearrange("(cc p) n -> p cc n", p=128))
            wq2 = cpool.tile([128, 2, C], FR, tag="wq2")
            nc.sync.dma_start(wq2[:], wq2_d[:].rearrange("(cc p) n -> p cc n", p=128).bitcast(FR))
            wk2p = cpool.tile([128, 2, C], FR, tag="wk2p")
            nc.sync.dma_start(wk2p[:], wk2p_d[:].rearrange("(cc p) n -> p cc n", p=128).bitcast(FR))
            wv2 = cpool.tile([128, 2, 64], FR, tag="wv2")
            nc.sync.dma_start(wv2[:], wv2_d[:].rearrange("(cc p) n -> p cc n", p=128).bitcast(FR))
            q2b = cpool.tile([128, C], FP, tag="q2b")
            nc.sync.dma_start(q2b[:], q2b_d[:])
            k2b = cpool.tile([128, C], FR, tag="k2b")
            nc.sync.dma_start(k2b[:], k2b_d[:].bitcast(FR))
            ones128 = cpool.tile([1, 128], FR, tag="ones128")
            nc.sync.dma_start(ones128[:], ones_d[:].bitcast(FR))
            v2b = cpool.tile([64, 1], FP, tag="v2b")
            nc.sync.dma_start(v2b[:], v2b_d[:])
            gam = cpool.tile([128, 2], FP, tag="gam")
            nc.sync.dma_start(gam[:], gam_d[:])
            ident = cpool.tile([128, 128], FP, tag="ident")
            make_identity(nc, ident[:])
            gtr = cpool.tile([128, 1], FP, tag="gtr")
            nc.vector.reciprocal(gtr[:], gam[:, 1:2])
            ident_gt = cpool.tile([128, 128], FR, tag="ident_gt")
            nc.vector.tensor_scalar_mul(ident_gt[:], ident[:], gtr[:, 0:1])
            actwarm = cpool.tile([128, 1], FP, tag="actwarm")
            nc.scalar.activation(out=actwarm[:], in_=gam[:, 0:1],
                                 func=Act.Exp)

            g_s = gam[:, 0:1]
            g_t = gam[:, 1:2]

            xs_tiles = []
            for p in range(2, L):
                for qt in range(4):
                    xst = xsp.tile([128, 2, 1024], FR, tag="xs")
                    nc.sync.dma_start(
                        xst[:],
                        xt_all[p, :, ds(1024 * qt, 1024)]
                        .rearrange("(cc p) n -> p cc n", p=128).bitcast(FR))
                    xs_tiles.append(xst)

            # ---- phase0: one closure per [64, 512] conv chunk ----
            def ph0_chunk(src_t, colofs, p, sg, e1="dve"):
                pm, dcp = p % 4, p // 4
                ps = ps512.tile([128, 512], FP, tag="mm512")
                pm64 = ps[0:64, :]
                nc.tensor.matmul(pm64, wv2[:, 0, :],
                                 src_t[:, 0, ds(colofs, 512)],
                                 start=True, stop=False)
                nc.tensor.matmul(pm64, wv2[:, 1, :],
                                 src_t[:, 1, ds(colofs, 512)],
                                 start=False, stop=True)
                nc.scalar.activation(
                    out=v2p[0][ds(32 * pm, 32), dcp, ts(sg, 512)],
                    in_=ps[0:32, :], func=Act.Identity, bias=v2b[0:32, 0:1])
                if e1 == "dve":
                    nc.vector.tensor_scalar_add(
                        v2p[1][ds(32 * pm, 32), dcp, ts(sg, 512)],
                        ps[32:64, :], v2b[32:64, 0:1])
                else:
                    nc.scalar.activation(
                        out=v2p[1][ds(32 * pm, 32), dcp, ts(sg, 512)],
                        in_=ps[32:64, :], func=Act.Identity,
                        bias=v2b[32:64, 0:1])

            stream_raw = []
            for idx, p in enumerate(range(2, L)):
                for qt in range(4):
                    for ch in range(2):
                        stream_raw.append(
                            (xs_tiles[4 * idx + qt], 512 * ch, p, 2 * qt + ch))
            res_raw = [(xt[j], 512 * sg, j, sg)
                       for j in range(2) for sg in range(8)]
            order = res_raw[:4] + stream_raw[:32]
            for i in range(12):
                order.append(res_raw[4 + i])
                order.append(stream_raw[32 + i])
            order.extend(stream_raw[44:])
            ph0_fillers = []
            for i, (t, o, pp, sg) in enumerate(order):
                e1 = "dve" if i < 40 else ("act" if i % 2 == 0 else "dve")
                ph0_fillers.append(
                    (lambda t=t, o=o, pp=pp, sg=sg, e1=e1:
                     ph0_chunk(t, o, pp, sg, e1)))
            filler_iter = iter(ph0_fillers)

            def fill(n=1):
                for _ in range(n):
                    f = next(filler_iter, None)
                    if f is None:
                        return
                    f()

            # ---- per-slice stages ----
            def loop1(j, fills):
                x = xt[j]
                scores = [psS.tile([128, C], FP, tag="scores",
                                   name=f"s1_{j}_{cc}") for cc in range(2)]
                for s1 in range(32):
                    pqk = ps512.tile([128, 512], FP, tag="mm512")
                    nc.tensor.matmul(pqk[:], x[:, 0, ts(s1, 128)],
                                     wqk[:, 0, :], start=True, stop=False)
                    nc.tensor.matmul(pqk[:], x[:, 1, ts(s1, 128)],
                                     wqk[:, 1, :], start=False, stop=True)
                    qk_sb = qkp.tile([128, 512], FR, tag="qk")
                    nc.vector.tensor_add(qk_sb[:], pqk[:], qkb[:])
                    fill(fills[s1])
                    first, last = s1 == 0, s1 == 31
                    for cc2 in range(2):
                        nc.tensor.matmul(scores[cc2][:],
                                         qk_sb[:, ts(cc2, 128)],
                                         qk_sb[:, 256:512],
                                         start=first, stop=last)
                return scores

            def softmax_t(scores, att, att_is_bf):
                # att: [128, 2, 256] tile (FP->FR-bitcast or BF)
                for cc in range(2):
                    mx = sm.tile([128, 1], FP, tag="mx")
                    nc.vector.reduce_max(mx[:], scores[cc][:],
                                         axis=mybir.AxisListType.X)
                    nmx = sm.tile([128, 1], FP, tag="nmx")
                    nc.vector.tensor_scalar_mul(nmx[:], mx[:], -1.0)
                    aexp = sm.tile([128, C], FP, tag="aexp")
                    ssum = sm.tile([128, 1], FP, tag="ssum")
                    nc.scalar.activation(out=aexp[:], in_=scores[cc][:],
                                         func=Act.Exp, bias=nmx[:],
                                         accum_out=ssum[:])
                    rs = sm.tile([128, 1], FP, tag="rs")
                    nc.vector.reciprocal(rs[:], ssum[:])
                    nc.vector.tensor_scalar_mul(aexp[:], aexp[:], rs[:])
                    pt = ps256.tile([128, 256], FP, tag="mm256",
                                    name=f"ptT{cc}")
                    for dc in range(2):
                        nc.tensor.transpose(pt[:, ts(dc, 128)],
                                            aexp[:, ts(dc, 128)],
                                            ident[:])
                    # one copy per cc: [128d(2dc), c-128] <- [128, 2*128]
                    nc.scalar.copy(att[:, :, ts(cc, 128)],
                                   pt[:].rearrange("p (a b) -> p a b", a=2))

            def vconv(j, s5, eng="act"):
                x = xt[j]
                vt = vtp.tile([128, 2, 512], FR, tag="vt")
                for dc in range(2):
                    pv = ps512.tile([128, 512], FP, tag="mm512")
                    nc.tensor.matmul(pv[:], wv[:, 0, ts(dc, 128)],
                                     x[:, 0, ts(s5, 512)],
                                     start=True, stop=False)
                    nc.tensor.matmul(pv[:], wv[:, 1, ts(dc, 128)],
                                     x[:, 1, ts(s5, 512)],
                                     start=False, stop=True)
                    if eng == "dve":
                        nc.vector.tensor_scalar_add(vt[:, dc, :], pv[:],
                                                    vb[:, dc, 0:1])
                    else:
                        nc.scalar.activation(out=vt[:, dc, :], in_=pv[:],
                                             func=Act.Identity,
                                             bias=vb[:, dc, 0:1])
                return vt

            def loop2(j, attnT, nfill, vt01, pre=None):
                # Phase A: spatial attention output, written to x1sw in the
                # H/W-swapped layout (u = w*64+h) via a strided stt output AP.
                # Phase B: q2 conv reads x1sw contiguously (= swapped q2),
                # k2 conv reads natural xt contiguously; scores2 accumulates
                # over the shared u enumeration.
                x = xt[j]
                scores2 = [psS.tile([128, C], FP, tag="scores",
                                    name=f"s2_{j}_{cc}") for cc in range(2)]
                vts = list(vt01)
                for s5 in range(NS512):
                    if pre is not None:
                        pre(s5)
                    vt = vts.pop(0)
                    for cc in range(2):
                        pcs = ps512.tile([128, 512], FP, tag="mm512")
                        nc.tensor.matmul(pcs[:], attnT[:, 0, ts(cc, 128)],
                                         vt[:, 0, :], start=True, stop=False)
                        nc.tensor.matmul(pcs[:], attnT[:, 1, ts(cc, 128)],
                                         vt[:, 1, :], start=False, stop=True)
                        nc.vector.scalar_tensor_tensor(
                            out=x1t[:].rearrange("p cc (w h) -> p cc w h", w=64)
                                [:, cc, :, ds(8 * s5, 8)],
                            in0=pcs[:].rearrange("p (h w) -> p w h", h=8),
                            scalar=g_s,
                            in1=x[:, cc, ts(s5, 512)]
                            .rearrange("p (h w) -> p w h", h=8),
                            op0=Alu.mult, op1=Alu.add)
                    if s5 + 2 < NS512:
                        vts.append(vconv(j, s5 + 2))
                    fill(nfill)
                for s1 in range(32):
                    pqk2 = ps512.tile([128, 512], FP, tag="mm512")
                    nc.tensor.matmul(pqk2[:, 0:256], x1t[:, 0, ts(s1, 128)],
                                     wq2[:, 0, :], start=True, stop=False)
                    nc.tensor.matmul(pqk2[:, 0:256], x1t[:, 1, ts(s1, 128)],
                                     wq2[:, 1, :], start=False, stop=True)
                    nc.tensor.matmul(pqk2[:, 256:512], x[:, 0, ts(s1, 128)],
                                     wk2p[:, 0, :], start=True, stop=False)
                    nc.tensor.matmul(pqk2[:, 256:512], x[:, 1, ts(s1, 128)],
                                     wk2p[:, 1, :], start=False, stop=False)
                    nc.tensor.matmul(pqk2[:, 256:512], ones128[:], k2b[0:1, :],
                                     start=False, stop=True)
                    qk2_sb = qkp.tile([128, 512], FR, tag="qk")
                    nc.vector.tensor_add(qk2_sb[:, 0:256], pqk2[:, 0:256],
                                         q2b[:])
                    nc.scalar.copy(qk2_sb[:, 256:512], pqk2[:, 256:512])
                    fill(nfill)
                    first, last = s1 == 0, s1 == 31
                    for cc2 in range(2):
                        nc.tensor.matmul(scores2[cc2][:],
                                         qk2_sb[:, ts(cc2, 128)],
                                         qk2_sb[:, 256:512],
                                         start=first, stop=last)
                return scores2

            def l3chunk(j, attn2T, s5):
                    for cc in range(2):
                        po = ps512.tile([128, 512], FP, tag="mm512")
                        nc.tensor.matmul(po[:], attn2T[:, 0, ts(cc, 128)],
                                         v2p[j][:, 0, ts(s5, 512)],
                                         start=True, stop=False)
                        nc.tensor.matmul(po[:], attn2T[:, 1, ts(cc, 128)],
                                         v2p[j][:, 1, ts(s5, 512)],
                                         start=False, stop=False)
                        nc.tensor.matmul(
                            po[:].rearrange("p (h w) -> p h w", h=8),
                            ident_gt[:],
                            x1t[:].rearrange("p cc (w h) -> p cc h w", w=64)
                            [:, cc, ds(8 * s5, 8), :],
                            start=False, stop=True)
                        x2c = x2p.tile([128, 512], BF, tag="x2c")
                        nc.scalar.activation(out=x2c[:], in_=po[:],
                                             func=Act.Identity,
                                             scale=g_t)
                        nc.sync.dma_start(
                            out_d[j, ds(128 * cc, 128), ts(s5, 512)], x2c[:])

            def loop3(j, attn2T):
                for s5 in range(NS512):
                    l3chunk(j, attn2T, s5)

            # ================= emission order =================
            scores_s0 = loop1(0, [0] * 32)
            vt01 = [vconv(0, 0), vconv(0, 1)]
            attnT0 = attp.tile([128, 2, C], FR, tag="attnT", name="attnT0")
            softmax_t(scores_s0, attnT0, False)
            fill(4)
            scores2_s0 = loop2(0, attnT0, 1, vt01)
            attn2T0 = attp.tile([128, 2, C], BF, tag="attnT2", name="attn2T0")
            softmax_t(scores2_s0, attn2T0, True)

            scores_s1 = loop1(1, [1] * 20 + [0] * 12)
            vt01b = [vconv(1, 0), vconv(1, 1)]
            attnT1 = attp.tile([128, 2, C], FR, tag="attnT", name="attnT1")
            softmax_t(scores_s1, attnT1, False)
            fill(4)

            fill(80)  # safety drain (normally empty)

            scores2_s1 = loop2(1, attnT1, 0, vt01b,
                               pre=lambda s5: l3chunk(0, attn2T0, s5))
            attn2T1 = attp.tile([128, 2, C], BF, tag="attnT2", name="attn2T1")
            softmax_t(scores2_s1, attn2T1, True)
            loop3(1, attn2T1)

    nc.compile()
    return nc


def _prep_core_inputs(x_s, w, k):
    """Host-side sharding for core k. x_s: (2,8,64,64,256) fp32. w: weights."""
    b, q = k // 4, k % 4
    l0 = (2 * k) % 8
    band = 64 * q
    rr = np.arange(C)
    # v2p row r = 128*(p//4) + 32*(p%4) + c_off holds phase0 position p
    # (actual l = (l0+p)%8) and band-channel c_off (0..31 within slice);
    # its true temporal column is 8*c_off + l.
    p_pos = 4 * (rr // 128) + (rr % 128) // 32
    dperm = 8 * (rr % 32) + ((l0 + p_pos) % 8)
    xb = x_s[b]  # (8,64,64,256)
    f32 = np.float32
    return {
        "xt_all": np.ascontiguousarray(np.stack(
            [xb[(l0 + p) % 8].transpose(2, 0, 1).reshape(C, HW)
             for p in range(L)]), f32),
        "wqk": np.ascontiguousarray(
            np.concatenate([w["sq_w"].T, w["sk_w"].T], axis=1), f32),
        "wv": np.ascontiguousarray(w["sv_w"].T, f32),
        "wq2": np.ascontiguousarray(w["tq_w"].T, f32),
        "wk2p": np.ascontiguousarray(w["tk_w"][dperm].T, f32),
        "wv2": np.ascontiguousarray(w["tv_w"][band:band + 64].T, f32),
        "qk_bias": np.ascontiguousarray(np.broadcast_to(
            np.concatenate([w["sq_b"], w["sk_b"]]), (128, 512)), f32),
        "v_bias": np.ascontiguousarray(w["sv_b"].reshape(C, 1), f32),
        "q2_bias": np.ascontiguousarray(np.broadcast_to(w["tq_b"], (128, C)), f32),
        "k2_bias": np.ascontiguousarray(
            np.broadcast_to(w["tk_b"][dperm], (128, C)), f32),
        "v2_bias": np.ascontiguousarray(
            w["tv_b"][band:band + 64].reshape(64, 1), f32),
        "gammas": np.ascontiguousarray(np.broadcast_to(
            np.stack([w["s_gamma"][0], w["t_gamma"][0]]), (128, 2)), f32),
        "ones_row": np.ones((1, 128), f32),
    }


def kernel(**inputs):
    x = np.asarray(inputs["x"], np.float32)
    x_s = np.ascontiguousarray(x[..., :C])
    wnames = ["sq_w", "sq_b", "sk_w", "sk_b", "sv_w", "sv_b",
              "tq_w", "tq_b", "tk_w", "tk_b", "tv_w", "tv_b",
              "s_gamma", "t_gamma"]
    w = {n: np.asarray(inputs[n], np.float32) for n in wnames}

    if "nc" not in _CACHE:
        _CACHE["nc"] = build_program()
    nc = _CACHE["nc"]

    in_maps = [_prep_core_inputs(x_s, w, k) for k in range(8)]
    res = run_bass_kernel_spmd(nc, in_maps, core_ids=list(range(8)))

    out = np.empty((B, L, H, W, C), np.float32)
    for k in range(8):
        o = np.asarray(res.results[k]["out"], np.float32)  # (2,256,4096) c-major bf16
        for j in range(2):
            i = 2 * k + j
            out[i // 8, i % 8] = o[j].reshape(C, H, W).transpose(1, 2, 0)
    return out


if __name__ == "__main__":
    import reference as ref
    inputs = {kk: np.asarray(v) for kk, v in ref.setup_inputs().items()}
    expected = np.asarray(ref.reference(**inputs))
    got = kernel(**inputs)
    err = np.abs(got - expected)
    rel = err.max() / np.abs(expected).max()
    print("abs max err:", err.max(), " rel:", float(rel))

